# revision 33
# baseline (speedup 1.0000x reference)
"""Sparse multi-head self-attention on 8 trn2 NeuronCores.

Problem: B=4, S=2048, D=768, H=12 heads of 64; only the <=512 keys selected by
`uniform_set` (and not padding-masked) participate in attention.

Sharding: core = 2*b + hg  (b = batch 0..3, hg = head-group 0..1, 6 heads each,
Megatron-style column-sharded Wq/Wk/Wv + row-sharded Wo).  Each core computes a
partial output [S, D] for its batch from its 6 heads; host sums the two
head-group partials per batch.

Device algorithm (per core).  Heads are processed as 3 pairs; the pair's two
heads live on partition halves 0:64 / 64:128 so their K=64 score matmuls
row-tile onto different PE quadrant rows and run concurrently.

  Qt[dout, s] = WqT^T . XT      Kt[dout, k] = WkT^T . KselT
  V' = VselT^T . WvT  stored as vb[k, c, j, q, 0:64]; vb[.., 64:128] = key mask
      (1.0 real key, 0.0 padded) replicated 64x (DMA'd pre-built from host)
  scoresT[k, s] per head; no bias needed: masked/padded keys have K=V=0 and
      mask=0, so exp(0)=1 contributes nothing to numerator (V row 0) or
      denominator (mask 0).
  expT = exp(scoresT)  - one ACT per (pair, c-half) over 4 PSUM banks
  ctx' = [V | mask]^T . expT  -> rows 0:64 = ctx, row 64 = sum of exp
  1/sum = exp(-ln(sum)) batched for the pair's 2 heads (rows 0/32, one
      pinned Exp+Ln table set); gpsimd partition_broadcast to 64 rows;
      ctxT = ctx'[0:64] * bcast(1/sum) on DVE
  out partial[s_chunk, d] = ctxT^T . WoT   -> bf16 -> DRAM

Biases: bq asserted 0.  bk shifts scores by a per-query constant (softmax
invariant).  bv and bo applied exactly on the host: out += bo + Wo @ bv.
"""

import os

import numpy as np

DEBUG = os.environ.get("KDBG") == "1"

B, S, D, H, HD = 4, 2048, 768, 12, 64
HG = 2            # head groups (tensor parallel)
HPG = H // HG     # 6 heads per group
DG = HPG * HD     # 384 projection dims per group
NK = 512          # padded count of selected keys
P = 128
KC = D // P       # 6 contraction chunks over model dim
MC = DG // P      # 3 head pairs per group
SC = NK // P      # 4 selected-key chunks
SQT = 512         # query-tile (moving free dim)
NSQT = S // SQT   # 4

_CACHE = {}


def _build_bass():
    import concourse.mybir as mybir
    import concourse.tile as tile
    from concourse import bacc

    f32 = mybir.dt.float32
    bf16 = mybir.dt.bfloat16
    EXP = mybir.ActivationFunctionType.Exp
    LN = mybir.ActivationFunctionType.Ln

    nc = bacc.Bacc("TRN2", name="sparse_mha")

    # all inputs arrive pre-rearranged from the host ([partition, chunk, m])
    # so every DMA is a fully contiguous per-partition stream
    xt_d = nc.dram_tensor("xt", [P, KC, S], bf16, kind="ExternalInput")
    kselt_d = nc.dram_tensor("kselt", [P, KC, NK], bf16, kind="ExternalInput")
    vselt_d = nc.dram_tensor("vselt", [P, KC, NK], bf16, kind="ExternalInput")
    wqt_d = nc.dram_tensor("wqt", [P, KC, DG], bf16, kind="ExternalInput")
    wkt_d = nc.dram_tensor("wkt", [P, KC, DG], bf16, kind="ExternalInput")
    wvt_d = nc.dram_tensor("wvt", [P, KC, DG], bf16, kind="ExternalInput")
    wot_d = nc.dram_tensor("wot", [P, MC, D], bf16, kind="ExternalInput")
    kmask_d = nc.dram_tensor("kmask64", [SC, P, HPG, 1], bf16, kind="ExternalInput")
    out_d = nc.dram_tensor("out", [S, D], bf16, kind="ExternalOutput")
    if DEBUG:
        dbg_ktp_d = nc.dram_tensor("dbg_ktp", [P, MC, NK], bf16, kind="ExternalOutput")
        dbg_qt_d = nc.dram_tensor("dbg_qt", [P, MC, S], bf16, kind="ExternalOutput")
        dbg_vb_d = nc.dram_tensor("dbg_vb", [P, SC, MC, 2, 66], bf16, kind="ExternalOutput")
        dbg_ep_d = nc.dram_tensor("dbg_ep", [P, SC, 2, SQT], bf16, kind="ExternalOutput")
        dbg_ctxt_d = nc.dram_tensor("dbg_ctxt", [P, MC, SQT], bf16, kind="ExternalOutput")
        dbg_xt_d = nc.dram_tensor("dbg_xt", [P, KC, S], bf16, kind="ExternalOutput")

    xt_r = xt_d
    kselt_r = kselt_d
    vselt_r = vselt_d

    with tile.TileContext(nc) as tc:
        with (
            tc.tile_pool(name="inputs", bufs=1) as inputs,
            tc.tile_pool(name="persist", bufs=1) as persist,
            tc.tile_pool(name="ep", bufs=4) as ep_pool,
            tc.tile_pool(name="cx", bufs=2) as cx_pool,
            tc.tile_pool(name="nrm", bufs=8) as nrm_pool,
            tc.tile_pool(name="ob", bufs=4) as ob_pool,
            tc.tile_pool(name="ps_sc", bufs=2, space="PSUM") as ps_sc,
            tc.tile_pool(name="ps_cx", bufs=3, space="PSUM") as ps_cx,
            tc.tile_pool(name="ps_pj", bufs=1, space="PSUM") as ps_pj,
        ):
            # ---- input loads: K path first, spread across engine queues so
            # issue costs don't serialize and the PE can start ~1.5us in ----
            wkt = inputs.tile([P, KC, DG], bf16, tag="wkt")
            kselt = inputs.tile([P, KC, NK], bf16, tag="kselt")
            wvt = inputs.tile([P, KC, DG], bf16, tag="wvt")
            vselt = inputs.tile([P, KC, NK], bf16, tag="vselt")
            wqt = inputs.tile([P, KC, DG], bf16, tag="wqt")
            xt = inputs.tile([P, KC, S], bf16, tag="xt")

            wot = persist.tile([P, MC, D], bf16, tag="wot")
            # vb: [keys, c, pair, parity, 0:64 V | col 64 = key mask]
            vb = persist.tile([P, SC, MC, 2, 66], bf16, tag="vb")

            # Three ~110GB/s queues (one per issuing engine), ordered so the
            # K-path (wkt+kselt) and Q-path (wqt+xt tile 0) land first; the
            # rest streams in behind the already-running pipeline.
            wkt_r = wkt_d
            wvt_r = wvt_d
            wqt_r = wqt_d
            wot_r = wot_d
            sq_of = lambda t: slice(t * SQT, (t + 1) * SQT)
            # critical path first on every queue: scores(pair 0) needs
            # kselt+wkt (-> ktp) and wqt+xt tile 0 (-> qt[:, :, 0:512])
            nc.scalar.dma_start(wqt, wqt_r[:, :, :])
            nc.scalar.dma_start(wkt[:, 3:6, :], wkt_r[:, 3:6, :])
            nc.sync.dma_start(kselt[:, 0:3, :], kselt_r[:, 0:3, :])
            nc.sync.dma_start(kselt[:, 3:6, :], kselt_r[:, 3:6, :])
            nc.gpsimd.dma_start(xt[:, :, sq_of(0)], xt_r[:, :, sq_of(0)])
            nc.gpsimd.dma_start(wkt[:, 0:3, :], wkt_r[:, 0:3, :])
            # second wave
            nc.scalar.dma_start(wvt, wvt_r[:, :, :])
            nc.scalar.dma_start(xt[:, 0:3, sq_of(2)], xt_r[:, 0:3, sq_of(2)])
            for c in range(SC):
                nc.scalar.dma_start(vb[:, c, :, :, HD : HD + 1], kmask_d[c])
            nc.scalar.dma_start(xt[:, 3:6, sq_of(2)], xt_r[:, 3:6, sq_of(2)])
            nc.sync.dma_start(xt[:, 0:3, sq_of(1)], xt_r[:, 0:3, sq_of(1)])
            nc.sync.dma_start(xt[:, 3:6, sq_of(1)], xt_r[:, 3:6, sq_of(1)])
            nc.sync.dma_start(wot, wot_r[:, :, :])
            nc.gpsimd.dma_start(vselt, vselt_r[:, :, :])
            nc.gpsimd.dma_start(xt[:, 0:3, sq_of(3)], xt_r[:, 0:3, sq_of(3)])
            nc.gpsimd.dma_start(xt[:, 3:6, sq_of(3)], xt_r[:, 3:6, sq_of(3)])

            ktp = persist.tile([P, MC, NK], bf16, tag="ktp")
            qt = persist.tile([P, MC, S], bf16, tag="qt")

            # staging rows for the batched 1/sum: rows 0 (q=0) and 32 (q=1);
            # rows 1..31 must be finite for the batched Ln -> memset once
            sums_a = persist.tile([P, SQT], f32, tag="sums_a")
            sums_b = persist.tile([P, SQT], f32, tag="sums_b")
            nc.gpsimd.memset(sums_a[0:33, :], 1.0)
            nc.gpsimd.memset(sums_b[0:33, :], 1.0)
            sums_ab = [sums_a, sums_b]

            # ---- Q projection of tile t (m-outer, one PSUM bank) ----
            def q_proj(t, m):
                sq = slice(t * SQT, (t + 1) * SQT)
                ps = ps_cx.tile([P, SQT], f32, tag="cx", name=f"qp{t}_{m}")
                for i in range(KC):
                    nc.tensor.matmul(
                        ps,
                        lhsT=wqt[:, i, m * P : (m + 1) * P],
                        rhs=xt[:, i, sq],
                        start=(i == 0),
                        stop=(i == KC - 1),
                    )
                nc.vector.tensor_copy(qt[:, m, sq], ps)

            # Qp(0) first: it gates scores(pair 0) and nothing blocks it
            for m in range(MC):
                q_proj(0, m)

            # ---- K projection, i-outer ----
            kps0 = ps_sc.tile([P, 2, SQT], f32, tag="sc", name="kps0")
            kps1 = ps_cx.tile([P, SQT], f32, tag="cx", name="kps1")
            for i in range(KC):
                st, sp = i == 0, i == KC - 1
                nc.tensor.matmul(kps0[:, 0, :], lhsT=wkt[:, i, 0:P], rhs=kselt[:, i, :], start=st, stop=sp)
                nc.tensor.matmul(kps0[:, 1, :], lhsT=wkt[:, i, P : 2 * P], rhs=kselt[:, i, :], start=st, stop=sp)
                nc.tensor.matmul(kps1, lhsT=wkt[:, i, 2 * P : 3 * P], rhs=kselt[:, i, :], start=st, stop=sp)
            nc.vector.tensor_copy(ktp[:, 0, :], kps0[:, 0, :])
            nc.vector.tensor_copy(ktp[:, 1, :], kps0[:, 1, :])
            nc.vector.tensor_copy(ktp[:, 2, :], kps1)

            # ---- V projection, i-outer, emitted as a slot-0 filler so late
            # vselt never head-blocks the score/exp pipeline ----
            def emit_vproj():
                vps1 = ps_cx.tile([P, SQT], f32, tag="cx", name="vps1")
                vps2 = ps_cx.tile([P, SQT], f32, tag="cx", name="vps2")
                vps3 = ps_cx.tile([P, SQT], f32, tag="cx", name="vps3")
                vps4 = ps_pj.tile([P, SQT], f32, tag="pj", name="vps4")
                vtgt = [vps1[:, 0:DG], vps2[:, 0:DG], vps3[:, 0:DG], vps4[:, 0:DG]]
                for i in range(KC):
                    st, sp = i == 0, i == KC - 1
                    for c in range(SC):
                        nc.tensor.matmul(
                            vtgt[c],
                            lhsT=vselt[:, i, c * P : (c + 1) * P],
                            rhs=wvt[:, i, :],
                            start=st,
                            stop=sp,
                        )
                for c in range(SC):
                    nc.vector.tensor_copy(
                        vb[:, c, :, :, 0:HD],
                        vtgt[c].rearrange("p (j q d) -> p j q d", j=MC, q=2),
                    )

            # ---- steady state: pipeline over pair-slots Pidx = 3t + j ----
            NP = NSQT * MC  # 12 pair slots
            ep_of = {}
            ctxt_of = {}
            obuf_of = {}

            def emit_scores(Pidx):
                t, j = Pidx // MC, Pidx % MC
                sq = slice(t * SQT, (t + 1) * SQT)
                ept = ep_pool.tile([P, SC, 2, SQT], bf16, tag="ep", name=f"ep{Pidx}")
                ep_of[Pidx] = ept
                for ch in range(SC):  # one 2-bank tile per c-chunk
                    sc = ps_sc.tile([P, 2, SQT], f32, tag="sc", name=f"sc{Pidx}_{ch}")
                    for q in range(2):
                        nc.tensor.matmul(
                            sc[:, q, :],
                            lhsT=ktp[64 * q : 64 * q + 64, j, ch * P : (ch + 1) * P],
                            rhs=qt[64 * q : 64 * q + 64, j, sq],
                            start=True,
                            stop=True,
                        )
                    nc.scalar.activation(out=ept[:, ch, :, :], in_=sc, func=EXP)
                    yield ch

            pc_of = {}

            def emit_ctx_half(Pp, q):
                # ctx' matmul for head (pair jp, parity q); M=65: rows 0:64
                # ctx, row 64 = sum of exp over unmasked keys
                if not (0 <= Pp < NP):
                    return
                tp, jp = Pp // MC, Pp % MC
                if jp == 0 and q == 0:
                    ctxt_of[tp] = cx_pool.tile([P, MC, SQT], bf16, tag="ctxt", name=f"ctxt{tp}")
                ept = ep_of[Pp]
                pc = ps_cx.tile([P, SQT], f32, tag="cx", name=f"cx{Pp}_{q}")
                pc_of[(Pp, q)] = pc
                for c in range(SC):
                    nc.tensor.matmul(
                        pc[: HD + 1, :],
                        lhsT=vb[:, c, jp, q, 0 : HD + 1],
                        rhs=ept[:, c, q, :],
                        start=(c == 0),
                        stop=(c == SC - 1),
                    )
                nc.vector.tensor_copy(
                    sums_ab[Pp % 2][32 * q : 32 * q + 1, :], pc[HD : HD + 1, :]
                )
                if DEBUG and Pp == 0 and q == 1:
                    nc.sync.dma_start(dbg_ep_d[:, :, :, :], ep_of[0])

            rs_of = {}

            def emit_norm_act(Pp):
                # batched 1/sums = exp(-ln(sums)) for the pair's two heads
                # (rows 0 and 32); emitted between the two exp ACTs so it
                # fills the scalar queue while the PE refills score banks
                if not (0 <= Pp < NP):
                    return
                s2 = sums_ab[Pp % 2]
                ls = nrm_pool.tile([P, SQT], f32, tag="ls", name=f"ls{Pp}")
                rs = nrm_pool.tile([P, SQT], f32, tag="rs", name=f"rs{Pp}")
                nc.scalar.activation(out=ls[0:33, :], in_=s2[0:33, :], func=LN)
                nc.scalar.activation(out=rs[0:33, :], in_=ls[0:33, :], func=EXP, scale=-1.0)
                rs_of[Pp] = rs

            def emit_norm_finish(Pp):
                # gpsimd partition_broadcast replicates each head's 1/sum row
                # to 64 SBUF rows (src must sit at absolute partition 0: q=1's
                # row 32 is DMA'd down first, from gpsimd's own queue so the
                # chain never touches the sync engine); then one fused DVE
                # multiply+cast produces ctxt
                if not (0 <= Pp < NP):
                    return
                tp, jp = Pp // MC, Pp % MC
                rs = rs_of.pop(Pp)
                for q in range(2):
                    pc = pc_of.pop((Pp, q))
                    if q == 0:
                        rrow = rs[0:1, :]
                    else:
                        r1 = nrm_pool.tile([1, SQT], f32, tag="r1", name=f"r1_{Pp}")
                        nc.sync.dma_start(r1, rs[32:33, :])
                        rrow = r1
                    rb = nrm_pool.tile([HD, SQT], f32, tag="rb", name=f"rb{Pp}_{q}")
                    nc.gpsimd.partition_broadcast(rb, rrow)
                    nc.vector.tensor_mul(
                        ctxt_of[tp][64 * q : 64 * q + 64, jp, :],
                        pc[0:HD, :],
                        rb[0:HD, :],
                    )

            def emit_outproj_chunk(tp, mq, n, use_sc=False):
                sq0 = tp * SQT + mq * P
                if n == 0:
                    obuf_of[(tp, mq)] = ob_pool.tile([P, D], bf16, tag="ob", name=f"ob{tp}_{mq}")
                if use_sc:
                    # drain slots: the score pool is idle, borrow it so the
                    # tail out-projections double-buffer
                    ps = ps_sc.tile([P, 2, SQT], f32, tag="sc", name=f"op{tp}_{mq}_{n}")[:, 0, :]
                else:
                    ps = ps_pj.tile([P, SQT], f32, tag="pj", name=f"op{tp}_{mq}_{n}")
                for jj in range(MC):
                    nc.tensor.matmul(
                        ps[:, 0:DG],
                        lhsT=ctxt_of[tp][:, jj, mq * P : (mq + 1) * P],
                        rhs=wot[:, jj, n * DG : (n + 1) * DG],
                        start=(jj == 0),
                        stop=(jj == MC - 1),
                    )
                nc.vector.tensor_copy(obuf_of[(tp, mq)][:, n * DG : (n + 1) * DG], ps[:, 0:DG])
                if n == 1:
                    nc.gpsimd.dma_start(out_d[sq0 : sq0 + P, :], obuf_of[(tp, mq)])

            # filler schedule: position -> list of work closures
            filler = {Pi: [] for Pi in range(NP + 5)}
            filler[0].append(("vp",))
            for t in range(NSQT - 1):
                for m in range(MC):
                    filler[MC * t + m].append(("qp", t + 1, m))
            for tp in range(NSQT):
                base = MC * tp + 4
                filler[base].extend([("op", tp, 0, 0), ("op", tp, 0, 1), ("op", tp, 1, 0)])
                filler[base + 1].extend([("op", tp, 1, 1), ("op", tp, 2, 0), ("op", tp, 2, 1)])
                filler[base + 2].extend([("op", tp, 3, 0), ("op", tp, 3, 1)])

            def run_filler(Pidx):
                for item in filler.get(Pidx, []):
                    if item[0] == "qp":
                        q_proj(item[1], item[2])
                    elif item[0] == "vp":
                        emit_vproj()
                    else:
                        emit_outproj_chunk(item[1], item[2], item[3], use_sc=Pidx >= NP)

            for Pidx in range(NP + 5):
                if Pidx < NP:
                    gen = emit_scores(Pidx)
                    next(gen)  # ch 0
                    next(gen)  # ch 1
                    emit_ctx_half(Pidx - 2, 0)
                    next(gen)  # ch 2
                    emit_ctx_half(Pidx - 2, 1)
                    emit_norm_act(Pidx - 2)
                    for _ in gen:  # ch 3
                        pass
                    emit_norm_finish(Pidx - 2)
                else:
                    emit_ctx_half(Pidx - 2, 0)
                    emit_ctx_half(Pidx - 2, 1)
                    emit_norm_act(Pidx - 2)
                    emit_norm_finish(Pidx - 2)
                run_filler(Pidx)
                if DEBUG and Pidx == NP + 3:
                    nc.sync.dma_start(dbg_xt_d[:, :, :], xt)
                if DEBUG and Pidx == 4:
                    nc.sync.dma_start(dbg_ktp_d[:, :, :], ktp)
                    nc.sync.dma_start(dbg_qt_d[:, :, :], qt)
                    nc.sync.dma_start(dbg_vb_d[:, :, :, :, :], vb)
                    nc.sync.dma_start(dbg_ctxt_d[:, :, :], ctxt_of[0])

    # Pin Exp and Ln to the one table set that holds both so the scalar
    # engine never reloads activation tables when alternating exp(scores)
    # with the ln/exp reciprocal.
    _orig_tables = bacc.get_activation_tables

    def _pinned_tables(arch):
        tabs = {k: set(v) for k, v in _orig_tables(arch).items()}
        for name, fns in tabs.items():
            if name != "natural_log_exp_and_others":
                fns.discard(EXP)
                fns.discard(LN)
        return tabs

    bacc.get_activation_tables = _pinned_tables
    try:
        nc.compile()
    finally:
        bacc.get_activation_tables = _orig_tables
    return nc


def _get_nc():
    if "nc" not in _CACHE:
        _CACHE["nc"] = _build_bass()
    return _CACHE["nc"]


def kernel(query, key, value, mask, uniform_set, Wq, bq, Wk, bk, Wv, bv, Wo, bo):
    import ml_dtypes
    from concourse import bass_utils

    bft = ml_dtypes.bfloat16

    query = np.asarray(query, dtype=np.float32)
    key = np.asarray(key, dtype=np.float32)
    value = np.asarray(value, dtype=np.float32)
    mask = np.asarray(mask, dtype=np.float32)
    us = np.asarray(uniform_set).astype(bool)
    Wq = np.asarray(Wq, dtype=np.float32)
    Wk = np.asarray(Wk, dtype=np.float32)
    Wv = np.asarray(Wv, dtype=np.float32)
    Wo = np.asarray(Wo, dtype=np.float32)
    bq = np.asarray(bq, dtype=np.float32)
    bk = np.asarray(bk, dtype=np.float32)
    bv = np.asarray(bv, dtype=np.float32)
    bo = np.asarray(bo, dtype=np.float32)
    assert np.all(bq == 0.0), "kernel assumes bq == 0 (reference generates zeros)"

    nc = _get_nc()

    scale = 1.0 / float(HD) ** 0.5

    def chunked(a):
        # [(o p), m] -> [p, o, m] contiguous (identity-layout device DMA)
        o = a.shape[0] // P
        return np.ascontiguousarray(a.reshape(o, P, a.shape[1]).transpose(1, 0, 2)).astype(bft)

    wqt_g = [chunked(Wq.T[:, g * DG : (g + 1) * DG] * scale) for g in range(HG)]
    wkt_g = [chunked(Wk.T[:, g * DG : (g + 1) * DG]) for g in range(HG)]
    wvt_g = [chunked(Wv.T[:, g * DG : (g + 1) * DG]) for g in range(HG)]
    wot_g = [chunked(Wo.T[g * DG : (g + 1) * DG, :]) for g in range(HG)]

    in_maps = []
    for b in range(B):
        keep = us & (mask[b, 0, 0] >= 0)
        idx = np.nonzero(keep)[0]
        n = len(idx)
        assert 0 < n <= NK, f"selected key count {n} unsupported"
        kselt = np.zeros((D, NK), np.float32)
        kselt[:, :n] = key[b][idx].T
        kselt = chunked(kselt)
        vselt = np.zeros((D, NK), np.float32)
        vselt[:, :n] = value[b][idx].T
        vselt = chunked(vselt)
        kmask = np.zeros((NK,), np.float32)
        kmask[:n] = 1.0
        # [SC, P, HPG, HD]: key k = c*128 + p, replicated over (head, hd)
        kmask64 = np.ascontiguousarray(
            np.broadcast_to(kmask.reshape(SC, P, 1, 1), (SC, P, HPG, 1))
        ).astype(bft)
        xt = chunked(query[b].T)
        for g in range(HG):
            in_maps.append(
                {
                    "xt": xt,
                    "kselt": kselt,
                    "vselt": vselt,
                    "wqt": wqt_g[g],
                    "wkt": wkt_g[g],
                    "wvt": wvt_g[g],
                    "wot": wot_g[g],
                    "kmask64": kmask64,
                }
            )

    res = bass_utils.run_bass_kernel_spmd(nc, in_maps, core_ids=list(range(B * HG)))
    outs = [np.asarray(m["out"], dtype=np.float32) for m in res.results]

    corr = (bo + Wo @ bv).astype(np.float32)
    out = np.empty((B, S, D), np.float32)
    for b in range(B):
        out[b] = outs[HG * b] + outs[HG * b + 1] + corr
    return out


# revision 35
# speedup vs baseline: 1.0084x; 1.0084x over previous
"""Sparse multi-head self-attention on 8 trn2 NeuronCores.

Problem: B=4, S=2048, D=768, H=12 heads of 64; only the <=512 keys selected by
`uniform_set` (and not padding-masked) participate in attention.

Sharding: core = 2*b + hg  (b = batch 0..3, hg = head-group 0..1, 6 heads each,
Megatron-style column-sharded Wq/Wk/Wv + row-sharded Wo).  Each core computes a
partial output [S, D] for its batch from its 6 heads; host sums the two
head-group partials per batch.

Device algorithm (per core).  Heads are processed as 3 pairs; the pair's two
heads live on partition halves 0:64 / 64:128 so their K=64 score matmuls
row-tile onto different PE quadrant rows and run concurrently.

  Qt[dout, s] = WqT^T . XT      Kt[dout, k] = WkT^T . KselT
  V' = VselT^T . WvT  stored as vb[k, c, j, q, 0:64]; vb[.., 64:128] = key mask
      (1.0 real key, 0.0 padded) replicated 64x (DMA'd pre-built from host)
  scoresT[k, s] per head; no bias needed: masked/padded keys have K=V=0 and
      mask=0, so exp(0)=1 contributes nothing to numerator (V row 0) or
      denominator (mask 0).
  expT = exp(scoresT)  - one ACT per (pair, c-half) over 4 PSUM banks
  ctx' = [V | mask]^T . expT  -> rows 0:64 = ctx, row 64 = sum of exp
  1/sum = exp(-ln(sum)) batched for the pair's 2 heads (rows 0/32, one
      pinned Exp+Ln table set); gpsimd partition_broadcast to 64 rows;
      ctxT = ctx'[0:64] * bcast(1/sum) on DVE
  out partial[s_chunk, d] = ctxT^T . WoT   -> bf16 -> DRAM

Biases: bq asserted 0.  bk shifts scores by a per-query constant (softmax
invariant).  bv and bo applied exactly on the host: out += bo + Wo @ bv.
"""

import os

import numpy as np

DEBUG = os.environ.get("KDBG") == "1"

B, S, D, H, HD = 4, 2048, 768, 12, 64
HG = 2            # head groups (tensor parallel)
HPG = H // HG     # 6 heads per group
DG = HPG * HD     # 384 projection dims per group
NK = 512          # padded count of selected keys
P = 128
KC = D // P       # 6 contraction chunks over model dim
MC = DG // P      # 3 head pairs per group
SC = NK // P      # 4 selected-key chunks
SQT = 512         # query-tile (moving free dim)
NSQT = S // SQT   # 4

_CACHE = {}


def _build_bass():
    import concourse.mybir as mybir
    import concourse.tile as tile
    from concourse import bacc

    f32 = mybir.dt.float32
    bf16 = mybir.dt.bfloat16
    EXP = mybir.ActivationFunctionType.Exp
    LN = mybir.ActivationFunctionType.Ln

    nc = bacc.Bacc("TRN2", name="sparse_mha")

    # all inputs arrive pre-rearranged from the host ([partition, chunk, m])
    # so every DMA is a fully contiguous per-partition stream
    xt_d = nc.dram_tensor("xt", [P, KC, S], bf16, kind="ExternalInput")
    kselt_d = nc.dram_tensor("kselt", [P, KC, NK], bf16, kind="ExternalInput")
    vselt_d = nc.dram_tensor("vselt", [P, KC, NK], bf16, kind="ExternalInput")
    wqt_d = nc.dram_tensor("wqt", [P, KC, DG], bf16, kind="ExternalInput")
    wkt_d = nc.dram_tensor("wkt", [P, KC, DG], bf16, kind="ExternalInput")
    wvt_d = nc.dram_tensor("wvt", [P, KC, DG], bf16, kind="ExternalInput")
    wot_d = nc.dram_tensor("wot", [P, MC, D], bf16, kind="ExternalInput")
    kmask_d = nc.dram_tensor("kmask64", [SC, P, HPG, 1], bf16, kind="ExternalInput")
    out_d = nc.dram_tensor("out", [S, D], bf16, kind="ExternalOutput")
    if DEBUG:
        dbg_ktp_d = nc.dram_tensor("dbg_ktp", [P, MC, NK], bf16, kind="ExternalOutput")
        dbg_qt_d = nc.dram_tensor("dbg_qt", [P, MC, S], bf16, kind="ExternalOutput")
        dbg_vb_d = nc.dram_tensor("dbg_vb", [P, SC, MC, 2, 66], bf16, kind="ExternalOutput")
        dbg_ep_d = nc.dram_tensor("dbg_ep", [P, SC, 2, SQT], bf16, kind="ExternalOutput")
        dbg_ctxt_d = nc.dram_tensor("dbg_ctxt", [P, MC, SQT], bf16, kind="ExternalOutput")
        dbg_xt_d = nc.dram_tensor("dbg_xt", [P, KC, S], bf16, kind="ExternalOutput")

    xt_r = xt_d
    kselt_r = kselt_d
    vselt_r = vselt_d

    with tile.TileContext(nc) as tc:
        with (
            tc.tile_pool(name="inputs", bufs=1) as inputs,
            tc.tile_pool(name="persist", bufs=1) as persist,
            tc.tile_pool(name="ep", bufs=4) as ep_pool,
            tc.tile_pool(name="cx", bufs=2) as cx_pool,
            tc.tile_pool(name="nrm", bufs=8) as nrm_pool,
            tc.tile_pool(name="ob", bufs=4) as ob_pool,
            tc.tile_pool(name="ps_sc", bufs=2, space="PSUM") as ps_sc,
            tc.tile_pool(name="ps_cx", bufs=3, space="PSUM") as ps_cx,
            tc.tile_pool(name="ps_pj", bufs=1, space="PSUM") as ps_pj,
        ):
            # ---- input loads: K path first, spread across engine queues so
            # issue costs don't serialize and the PE can start ~1.5us in ----
            wkt = inputs.tile([P, KC, DG], bf16, tag="wkt")
            kselt = inputs.tile([P, KC, NK], bf16, tag="kselt")
            wvt = inputs.tile([P, KC, DG], bf16, tag="wvt")
            vselt = inputs.tile([P, KC, NK], bf16, tag="vselt")
            wqt = inputs.tile([P, KC, DG], bf16, tag="wqt")
            xt = inputs.tile([P, KC, S], bf16, tag="xt")

            wot = persist.tile([P, MC, D], bf16, tag="wot")
            # vb: [keys, c, pair, parity, 0:64 V | col 64 = key mask]
            vb = persist.tile([P, SC, MC, 2, 66], bf16, tag="vb")

            # Three ~110GB/s queues (one per issuing engine), ordered so the
            # K-path (wkt+kselt) and Q-path (wqt+xt tile 0) land first; the
            # rest streams in behind the already-running pipeline.
            wkt_r = wkt_d
            wvt_r = wvt_d
            wqt_r = wqt_d
            wot_r = wot_d
            sq_of = lambda t: slice(t * SQT, (t + 1) * SQT)
            # critical path first on every queue: scores(pair 0) needs
            # kselt+wkt (-> ktp) and wqt+xt tile 0 (-> qt[:, :, 0:512])
            for ih in range(3):
                io = slice(2 * ih, 2 * ih + 2)
                nc.scalar.dma_start(wqt[:, io, :], wqt_r[:, io, :])
                nc.gpsimd.dma_start(xt[:, io, sq_of(0)], xt_r[:, io, sq_of(0)])
            nc.sync.dma_start(kselt[:, 0:3, :], kselt_r[:, 0:3, :])
            nc.sync.dma_start(kselt[:, 3:6, :], kselt_r[:, 3:6, :])
            nc.sync.dma_start(wkt, wkt_r[:, :, :])
            # second wave
            nc.scalar.dma_start(wvt, wvt_r[:, :, :])
            nc.scalar.dma_start(xt[:, 0:3, sq_of(2)], xt_r[:, 0:3, sq_of(2)])
            for c in range(SC):
                nc.scalar.dma_start(vb[:, c, :, :, HD : HD + 1], kmask_d[c])
            nc.scalar.dma_start(xt[:, 3:6, sq_of(2)], xt_r[:, 3:6, sq_of(2)])
            nc.sync.dma_start(xt[:, 0:3, sq_of(1)], xt_r[:, 0:3, sq_of(1)])
            nc.sync.dma_start(xt[:, 3:6, sq_of(1)], xt_r[:, 3:6, sq_of(1)])
            nc.sync.dma_start(wot, wot_r[:, :, :])
            nc.gpsimd.dma_start(vselt, vselt_r[:, :, :])
            nc.gpsimd.dma_start(xt[:, 0:3, sq_of(3)], xt_r[:, 0:3, sq_of(3)])
            nc.gpsimd.dma_start(xt[:, 3:6, sq_of(3)], xt_r[:, 3:6, sq_of(3)])

            ktp = persist.tile([P, MC, NK], bf16, tag="ktp")
            qt = persist.tile([P, MC, S], bf16, tag="qt")

            # staging rows for the batched 1/sum: rows 0 (q=0) and 32 (q=1);
            # rows 1..31 must be finite for the batched Ln -> memset once
            sums_a = persist.tile([P, SQT], f32, tag="sums_a")
            sums_b = persist.tile([P, SQT], f32, tag="sums_b")
            nc.gpsimd.memset(sums_a[0:33, :], 1.0)
            nc.gpsimd.memset(sums_b[0:33, :], 1.0)
            sums_ab = [sums_a, sums_b]

            # ---- Q projection of tile t (m-outer, one PSUM bank) ----
            def q_proj(t, m):
                sq = slice(t * SQT, (t + 1) * SQT)
                ps = ps_cx.tile([P, SQT], f32, tag="cx", name=f"qp{t}_{m}")
                for i in range(KC):
                    nc.tensor.matmul(
                        ps,
                        lhsT=wqt[:, i, m * P : (m + 1) * P],
                        rhs=xt[:, i, sq],
                        start=(i == 0),
                        stop=(i == KC - 1),
                    )
                nc.vector.tensor_copy(qt[:, m, sq], ps)

            # Qp(0) first, i-outer: its first matmul needs only the first
            # wqt/xt sub-chunks, so the PE starts ~4us in instead of ~19us
            q0ps = [ps_cx.tile([P, SQT], f32, tag="cx", name=f"q0ps{m}") for m in range(MC)]
            for i in range(KC):
                st, sp = i == 0, i == KC - 1
                for m in range(MC):
                    nc.tensor.matmul(
                        q0ps[m],
                        lhsT=wqt[:, i, m * P : (m + 1) * P],
                        rhs=xt[:, i, 0:SQT],
                        start=st,
                        stop=sp,
                    )
            for m in range(MC):
                nc.vector.tensor_copy(qt[:, m, 0:SQT], q0ps[m])

            # ---- K projection, i-outer ----
            kps0 = ps_sc.tile([P, 2, SQT], f32, tag="sc", name="kps0")
            kps1 = ps_cx.tile([P, SQT], f32, tag="cx", name="kps1")
            for i in range(KC):
                st, sp = i == 0, i == KC - 1
                nc.tensor.matmul(kps0[:, 0, :], lhsT=wkt[:, i, 0:P], rhs=kselt[:, i, :], start=st, stop=sp)
                nc.tensor.matmul(kps0[:, 1, :], lhsT=wkt[:, i, P : 2 * P], rhs=kselt[:, i, :], start=st, stop=sp)
                nc.tensor.matmul(kps1, lhsT=wkt[:, i, 2 * P : 3 * P], rhs=kselt[:, i, :], start=st, stop=sp)
            nc.vector.tensor_copy(ktp[:, 0, :], kps0[:, 0, :])
            nc.vector.tensor_copy(ktp[:, 1, :], kps0[:, 1, :])
            nc.vector.tensor_copy(ktp[:, 2, :], kps1)

            # ---- V projection, i-outer, emitted as a slot-0 filler so late
            # vselt never head-blocks the score/exp pipeline ----
            def emit_vproj():
                vps1 = ps_cx.tile([P, SQT], f32, tag="cx", name="vps1")
                vps2 = ps_cx.tile([P, SQT], f32, tag="cx", name="vps2")
                vps3 = ps_cx.tile([P, SQT], f32, tag="cx", name="vps3")
                vps4 = ps_pj.tile([P, SQT], f32, tag="pj", name="vps4")
                vtgt = [vps1[:, 0:DG], vps2[:, 0:DG], vps3[:, 0:DG], vps4[:, 0:DG]]
                for i in range(KC):
                    st, sp = i == 0, i == KC - 1
                    for c in range(SC):
                        nc.tensor.matmul(
                            vtgt[c],
                            lhsT=vselt[:, i, c * P : (c + 1) * P],
                            rhs=wvt[:, i, :],
                            start=st,
                            stop=sp,
                        )
                for c in range(SC):
                    nc.vector.tensor_copy(
                        vb[:, c, :, :, 0:HD],
                        vtgt[c].rearrange("p (j q d) -> p j q d", j=MC, q=2),
                    )

            # ---- steady state: pipeline over pair-slots Pidx = 3t + j ----
            NP = NSQT * MC  # 12 pair slots
            ep_of = {}
            ctxt_of = {}
            obuf_of = {}

            def emit_scores(Pidx):
                t, j = Pidx // MC, Pidx % MC
                sq = slice(t * SQT, (t + 1) * SQT)
                ept = ep_pool.tile([P, SC, 2, SQT], bf16, tag="ep", name=f"ep{Pidx}")
                ep_of[Pidx] = ept
                for ch in range(SC):  # one 2-bank tile per c-chunk
                    sc = ps_sc.tile([P, 2, SQT], f32, tag="sc", name=f"sc{Pidx}_{ch}")
                    for q in range(2):
                        nc.tensor.matmul(
                            sc[:, q, :],
                            lhsT=ktp[64 * q : 64 * q + 64, j, ch * P : (ch + 1) * P],
                            rhs=qt[64 * q : 64 * q + 64, j, sq],
                            start=True,
                            stop=True,
                        )
                    nc.scalar.activation(out=ept[:, ch, :, :], in_=sc, func=EXP)
                    yield ch

            pc_of = {}

            def emit_ctx_half(Pp, q):
                # ctx' matmul for head (pair jp, parity q); M=65: rows 0:64
                # ctx, row 64 = sum of exp over unmasked keys
                if not (0 <= Pp < NP):
                    return
                tp, jp = Pp // MC, Pp % MC
                if jp == 0 and q == 0:
                    ctxt_of[tp] = cx_pool.tile([P, MC, SQT], bf16, tag="ctxt", name=f"ctxt{tp}")
                ept = ep_of[Pp]
                pc = ps_cx.tile([P, SQT], f32, tag="cx", name=f"cx{Pp}_{q}")
                pc_of[(Pp, q)] = pc
                for c in range(SC):
                    nc.tensor.matmul(
                        pc[: HD + 1, :],
                        lhsT=vb[:, c, jp, q, 0 : HD + 1],
                        rhs=ept[:, c, q, :],
                        start=(c == 0),
                        stop=(c == SC - 1),
                    )
                nc.vector.tensor_copy(
                    sums_ab[Pp % 2][32 * q : 32 * q + 1, :], pc[HD : HD + 1, :]
                )
                if DEBUG and Pp == 0 and q == 1:
                    nc.sync.dma_start(dbg_ep_d[:, :, :, :], ep_of[0])

            rs_of = {}

            def emit_norm_act(Pp):
                # batched 1/sums = exp(-ln(sums)) for the pair's two heads
                # (rows 0 and 32); emitted between the two exp ACTs so it
                # fills the scalar queue while the PE refills score banks
                if not (0 <= Pp < NP):
                    return
                s2 = sums_ab[Pp % 2]
                ls = nrm_pool.tile([P, SQT], f32, tag="ls", name=f"ls{Pp}")
                rs = nrm_pool.tile([P, SQT], f32, tag="rs", name=f"rs{Pp}")
                nc.scalar.activation(out=ls[0:33, :], in_=s2[0:33, :], func=LN)
                nc.scalar.activation(out=rs[0:33, :], in_=ls[0:33, :], func=EXP, scale=-1.0)
                rs_of[Pp] = rs

            def emit_norm_finish(Pp):
                # gpsimd partition_broadcast replicates each head's 1/sum row
                # to 64 SBUF rows (src must sit at absolute partition 0: q=1's
                # row 32 is DMA'd down first, from gpsimd's own queue so the
                # chain never touches the sync engine); then one fused DVE
                # multiply+cast produces ctxt
                if not (0 <= Pp < NP):
                    return
                tp, jp = Pp // MC, Pp % MC
                rs = rs_of.pop(Pp)
                for q in range(2):
                    pc = pc_of.pop((Pp, q))
                    if q == 0:
                        rrow = rs[0:1, :]
                    else:
                        r1 = nrm_pool.tile([1, SQT], f32, tag="r1", name=f"r1_{Pp}")
                        nc.sync.dma_start(r1, rs[32:33, :])
                        rrow = r1
                    rb = nrm_pool.tile([HD, SQT], f32, tag="rb", name=f"rb{Pp}_{q}")
                    nc.gpsimd.partition_broadcast(rb, rrow)
                    nc.vector.tensor_mul(
                        ctxt_of[tp][64 * q : 64 * q + 64, jp, :],
                        pc[0:HD, :],
                        rb[0:HD, :],
                    )

            def emit_outproj_chunk(tp, mq, n, use_sc=False):
                sq0 = tp * SQT + mq * P
                if n == 0:
                    obuf_of[(tp, mq)] = ob_pool.tile([P, D], bf16, tag="ob", name=f"ob{tp}_{mq}")
                if use_sc:
                    # drain slots: the score pool is idle, borrow it so the
                    # tail out-projections double-buffer
                    ps = ps_sc.tile([P, 2, SQT], f32, tag="sc", name=f"op{tp}_{mq}_{n}")[:, 0, :]
                else:
                    ps = ps_pj.tile([P, SQT], f32, tag="pj", name=f"op{tp}_{mq}_{n}")
                for jj in range(MC):
                    nc.tensor.matmul(
                        ps[:, 0:DG],
                        lhsT=ctxt_of[tp][:, jj, mq * P : (mq + 1) * P],
                        rhs=wot[:, jj, n * DG : (n + 1) * DG],
                        start=(jj == 0),
                        stop=(jj == MC - 1),
                    )
                nc.vector.tensor_copy(obuf_of[(tp, mq)][:, n * DG : (n + 1) * DG], ps[:, 0:DG])
                if n == 1:
                    nc.gpsimd.dma_start(out_d[sq0 : sq0 + P, :], obuf_of[(tp, mq)])

            # filler schedule: position -> list of work closures
            filler = {Pi: [] for Pi in range(NP + 5)}
            filler[0].append(("vp",))
            for t in range(NSQT - 1):
                for m in range(MC):
                    filler[MC * t + m].append(("qp", t + 1, m))
            for tp in range(NSQT):
                base = MC * tp + 5
                filler[base].extend([("op", tp, 0, 0), ("op", tp, 0, 1), ("op", tp, 1, 0)])
                filler[base + 1].extend([("op", tp, 1, 1), ("op", tp, 2, 0), ("op", tp, 2, 1)])
                filler[base + 2].extend([("op", tp, 3, 0), ("op", tp, 3, 1)])

            def run_filler(Pidx):
                for item in filler.get(Pidx, []):
                    if item[0] == "qp":
                        q_proj(item[1], item[2])
                    elif item[0] == "vp":
                        emit_vproj()
                    else:
                        emit_outproj_chunk(item[1], item[2], item[3], use_sc=Pidx >= NP)

            for Pidx in range(NP + 5):
                if Pidx < NP:
                    gen = emit_scores(Pidx)
                    next(gen)  # ch 0
                    next(gen)  # ch 1
                    emit_ctx_half(Pidx - 2, 0)
                    next(gen)  # ch 2
                    emit_ctx_half(Pidx - 2, 1)
                    emit_norm_act(Pidx - 2)
                    for _ in gen:  # ch 3
                        pass
                    emit_norm_finish(Pidx - 2)
                else:
                    emit_ctx_half(Pidx - 2, 0)
                    emit_ctx_half(Pidx - 2, 1)
                    emit_norm_act(Pidx - 2)
                    emit_norm_finish(Pidx - 2)
                run_filler(Pidx)
                if DEBUG and Pidx == NP + 3:
                    nc.sync.dma_start(dbg_xt_d[:, :, :], xt)
                if DEBUG and Pidx == 4:
                    nc.sync.dma_start(dbg_ktp_d[:, :, :], ktp)
                    nc.sync.dma_start(dbg_qt_d[:, :, :], qt)
                    nc.sync.dma_start(dbg_vb_d[:, :, :, :, :], vb)
                    nc.sync.dma_start(dbg_ctxt_d[:, :, :], ctxt_of[0])

    # Pin Exp and Ln to the one table set that holds both so the scalar
    # engine never reloads activation tables when alternating exp(scores)
    # with the ln/exp reciprocal.
    _orig_tables = bacc.get_activation_tables

    def _pinned_tables(arch):
        tabs = {k: set(v) for k, v in _orig_tables(arch).items()}
        for name, fns in tabs.items():
            if name != "natural_log_exp_and_others":
                fns.discard(EXP)
                fns.discard(LN)
        return tabs

    bacc.get_activation_tables = _pinned_tables
    try:
        nc.compile()
    finally:
        bacc.get_activation_tables = _orig_tables
    return nc


def _get_nc():
    if "nc" not in _CACHE:
        _CACHE["nc"] = _build_bass()
    return _CACHE["nc"]


def kernel(query, key, value, mask, uniform_set, Wq, bq, Wk, bk, Wv, bv, Wo, bo):
    import ml_dtypes
    from concourse import bass_utils

    bft = ml_dtypes.bfloat16

    query = np.asarray(query, dtype=np.float32)
    key = np.asarray(key, dtype=np.float32)
    value = np.asarray(value, dtype=np.float32)
    mask = np.asarray(mask, dtype=np.float32)
    us = np.asarray(uniform_set).astype(bool)
    Wq = np.asarray(Wq, dtype=np.float32)
    Wk = np.asarray(Wk, dtype=np.float32)
    Wv = np.asarray(Wv, dtype=np.float32)
    Wo = np.asarray(Wo, dtype=np.float32)
    bq = np.asarray(bq, dtype=np.float32)
    bk = np.asarray(bk, dtype=np.float32)
    bv = np.asarray(bv, dtype=np.float32)
    bo = np.asarray(bo, dtype=np.float32)
    assert np.all(bq == 0.0), "kernel assumes bq == 0 (reference generates zeros)"

    nc = _get_nc()

    scale = 1.0 / float(HD) ** 0.5

    def chunked(a):
        # [(o p), m] -> [p, o, m] contiguous (identity-layout device DMA)
        o = a.shape[0] // P
        return np.ascontiguousarray(a.reshape(o, P, a.shape[1]).transpose(1, 0, 2)).astype(bft)

    wqt_g = [chunked(Wq.T[:, g * DG : (g + 1) * DG] * scale) for g in range(HG)]
    wkt_g = [chunked(Wk.T[:, g * DG : (g + 1) * DG]) for g in range(HG)]
    wvt_g = [chunked(Wv.T[:, g * DG : (g + 1) * DG]) for g in range(HG)]
    wot_g = [chunked(Wo.T[g * DG : (g + 1) * DG, :]) for g in range(HG)]

    in_maps = []
    for b in range(B):
        keep = us & (mask[b, 0, 0] >= 0)
        idx = np.nonzero(keep)[0]
        n = len(idx)
        assert 0 < n <= NK, f"selected key count {n} unsupported"
        kselt = np.zeros((D, NK), np.float32)
        kselt[:, :n] = key[b][idx].T
        kselt = chunked(kselt)
        vselt = np.zeros((D, NK), np.float32)
        vselt[:, :n] = value[b][idx].T
        vselt = chunked(vselt)
        kmask = np.zeros((NK,), np.float32)
        kmask[:n] = 1.0
        # [SC, P, HPG, HD]: key k = c*128 + p, replicated over (head, hd)
        kmask64 = np.ascontiguousarray(
            np.broadcast_to(kmask.reshape(SC, P, 1, 1), (SC, P, HPG, 1))
        ).astype(bft)
        xt = chunked(query[b].T)
        for g in range(HG):
            in_maps.append(
                {
                    "xt": xt,
                    "kselt": kselt,
                    "vselt": vselt,
                    "wqt": wqt_g[g],
                    "wkt": wkt_g[g],
                    "wvt": wvt_g[g],
                    "wot": wot_g[g],
                    "kmask64": kmask64,
                }
            )

    res = bass_utils.run_bass_kernel_spmd(nc, in_maps, core_ids=list(range(B * HG)))
    outs = [np.asarray(m["out"], dtype=np.float32) for m in res.results]

    corr = (bo + Wo @ bv).astype(np.float32)
    out = np.empty((B, S, D), np.float32)
    for b in range(B):
        out[b] = outs[HG * b] + outs[HG * b + 1] + corr
    return out


# revision 37
# speedup vs baseline: 1.0312x; 1.0226x over previous
"""Sparse multi-head self-attention on 8 trn2 NeuronCores.

Problem: B=4, S=2048, D=768, H=12 heads of 64; only the <=512 keys selected by
`uniform_set` (and not padding-masked) participate in attention.

Sharding: core = 2*b + hg  (b = batch 0..3, hg = head-group 0..1, 6 heads each,
Megatron-style column-sharded Wq/Wk/Wv + row-sharded Wo).  Each core computes a
partial output [S, D] for its batch from its 6 heads; host sums the two
head-group partials per batch.

Device algorithm (per core).  Heads are processed as 3 pairs; the pair's two
heads live on partition halves 0:64 / 64:128 so their K=64 score matmuls
row-tile onto different PE quadrant rows and run concurrently.

  Qt[dout, s] = WqT^T . XT      Kt[dout, k] = WkT^T . KselT
  V' = VselT^T . WvT  stored as vb[k, c, j, q, 0:64]; vb[.., 64:128] = key mask
      (1.0 real key, 0.0 padded) replicated 64x (DMA'd pre-built from host)
  scoresT[k, s] per head; no bias needed: masked/padded keys have K=V=0 and
      mask=0, so exp(0)=1 contributes nothing to numerator (V row 0) or
      denominator (mask 0).
  expT = exp(scoresT)  - one ACT per (pair, c-half) over 4 PSUM banks
  ctx' = [V | mask]^T . expT  -> rows 0:64 = ctx, row 64 = sum of exp
  1/sum = exp(-ln(sum)) batched for the pair's 2 heads (rows 0/32, one
      pinned Exp+Ln table set); gpsimd partition_broadcast to 64 rows;
      ctxT = ctx'[0:64] * bcast(1/sum) on DVE
  out partial[s_chunk, d] = ctxT^T . WoT   -> bf16 -> DRAM

Biases: bq asserted 0.  bk shifts scores by a per-query constant (softmax
invariant).  bv and bo applied exactly on the host: out += bo + Wo @ bv.
"""

import os

import numpy as np

DEBUG = os.environ.get("KDBG") == "1"

B, S, D, H, HD = 4, 2048, 768, 12, 64
HG = 2            # head groups (tensor parallel)
HPG = H // HG     # 6 heads per group
DG = HPG * HD     # 384 projection dims per group
NK = 512          # padded count of selected keys
P = 128
KC = D // P       # 6 contraction chunks over model dim
MC = DG // P      # 3 head pairs per group
SC = NK // P      # 4 selected-key chunks
SQT = 512         # query-tile (moving free dim)
NSQT = S // SQT   # 4

_CACHE = {}


def _build_bass():
    import concourse.mybir as mybir
    import concourse.tile as tile
    from concourse import bacc

    f32 = mybir.dt.float32
    bf16 = mybir.dt.bfloat16
    EXP = mybir.ActivationFunctionType.Exp
    LN = mybir.ActivationFunctionType.Ln

    nc = bacc.Bacc("TRN2", name="sparse_mha")

    xt_d = nc.dram_tensor("xt", [D, S], bf16, kind="ExternalInput")
    kselt_d = nc.dram_tensor("kselt", [D, NK], bf16, kind="ExternalInput")
    vselt_d = nc.dram_tensor("vselt", [D, NK], bf16, kind="ExternalInput")
    wqt_d = nc.dram_tensor("wqt", [D, DG], bf16, kind="ExternalInput")
    wkt_d = nc.dram_tensor("wkt", [D, DG], bf16, kind="ExternalInput")
    wvt_d = nc.dram_tensor("wvt", [D, DG], bf16, kind="ExternalInput")
    wot_d = nc.dram_tensor("wot", [DG, D], bf16, kind="ExternalInput")
    kmask_d = nc.dram_tensor("kmask64", [SC, P, HPG, 1], bf16, kind="ExternalInput")
    out_d = nc.dram_tensor("out", [S, D], bf16, kind="ExternalOutput")
    if DEBUG:
        dbg_ktp_d = nc.dram_tensor("dbg_ktp", [P, MC, NK], bf16, kind="ExternalOutput")
        dbg_qt_d = nc.dram_tensor("dbg_qt", [P, MC, S], bf16, kind="ExternalOutput")
        dbg_vb_d = nc.dram_tensor("dbg_vb", [P, SC, MC, 2, 66], bf16, kind="ExternalOutput")
        dbg_ep_d = nc.dram_tensor("dbg_ep", [P, SC, 2, SQT], bf16, kind="ExternalOutput")
        dbg_ctxt_d = nc.dram_tensor("dbg_ctxt", [P, MC, SQT], bf16, kind="ExternalOutput")
        dbg_xt_d = nc.dram_tensor("dbg_xt", [P, KC, S], bf16, kind="ExternalOutput")

    xt_r = xt_d.rearrange("(o p) m -> p o m", p=P)
    kselt_r = kselt_d.rearrange("(o p) m -> p o m", p=P)
    vselt_r = vselt_d.rearrange("(o p) m -> p o m", p=P)

    with tile.TileContext(nc) as tc:
        with (
            tc.tile_pool(name="inputs", bufs=1) as inputs,
            tc.tile_pool(name="persist", bufs=1) as persist,
            tc.tile_pool(name="ep", bufs=4) as ep_pool,
            tc.tile_pool(name="cx", bufs=2) as cx_pool,
            tc.tile_pool(name="nrm", bufs=8) as nrm_pool,
            tc.tile_pool(name="ob", bufs=4) as ob_pool,
            tc.tile_pool(name="ps_sc", bufs=2, space="PSUM") as ps_sc,
            tc.tile_pool(name="ps_cx", bufs=3, space="PSUM") as ps_cx,
            tc.tile_pool(name="ps_pj", bufs=1, space="PSUM") as ps_pj,
        ):
            # ---- input loads: K path first, spread across engine queues so
            # issue costs don't serialize and the PE can start ~1.5us in ----
            wkt = inputs.tile([P, KC, DG], bf16, tag="wkt")
            kselt = inputs.tile([P, KC, NK], bf16, tag="kselt")
            wvt = inputs.tile([P, KC, DG], bf16, tag="wvt")
            vselt = inputs.tile([P, KC, NK], bf16, tag="vselt")
            wqt = inputs.tile([P, KC, DG], bf16, tag="wqt")
            xt = inputs.tile([P, KC, S], bf16, tag="xt")

            wot = persist.tile([P, MC, D], bf16, tag="wot")
            # vb: [keys, c, pair, parity, 0:64 V | col 64 = key mask]
            vb = persist.tile([P, SC, MC, 2, 66], bf16, tag="vb")

            # Three ~110GB/s queues (one per issuing engine), ordered so the
            # K-path (wkt+kselt) and Q-path (wqt+xt tile 0) land first; the
            # rest streams in behind the already-running pipeline.
            wkt_r = wkt_d.rearrange("(o p) m -> p o m", p=P)
            wvt_r = wvt_d.rearrange("(o p) m -> p o m", p=P)
            wqt_r = wqt_d.rearrange("(o p) m -> p o m", p=P)
            wot_r = wot_d.rearrange("(o p) m -> p o m", p=P)
            sq_of = lambda t: slice(t * SQT, (t + 1) * SQT)
            # critical path first on every queue: scores(pair 0) needs
            # kselt+wkt (-> ktp) and wqt+xt tile 0 (-> qt[:, :, 0:512])
            nc.scalar.dma_start(wqt, wqt_r)
            nc.scalar.dma_start(wkt[:, 3:6, :], wkt_r[:, 3:6, :])
            nc.sync.dma_start(kselt[:, 0:3, :], kselt_r[:, 0:3, :])
            nc.sync.dma_start(kselt[:, 3:6, :], kselt_r[:, 3:6, :])
            nc.gpsimd.dma_start(xt[:, :, sq_of(0)], xt_r[:, :, sq_of(0)])
            nc.gpsimd.dma_start(wkt[:, 0:3, :], wkt_r[:, 0:3, :])
            # second wave
            nc.scalar.dma_start(wvt, wvt_r)
            nc.scalar.dma_start(xt[:, 0:3, sq_of(2)], xt_r[:, 0:3, sq_of(2)])
            for c in range(SC):
                nc.scalar.dma_start(vb[:, c, :, :, HD : HD + 1], kmask_d[c])
            nc.scalar.dma_start(xt[:, 3:6, sq_of(2)], xt_r[:, 3:6, sq_of(2)])
            nc.sync.dma_start(xt[:, 0:3, sq_of(1)], xt_r[:, 0:3, sq_of(1)])
            nc.sync.dma_start(xt[:, 3:6, sq_of(1)], xt_r[:, 3:6, sq_of(1)])
            nc.sync.dma_start(wot, wot_r)
            nc.gpsimd.dma_start(vselt, vselt_r)
            nc.gpsimd.dma_start(xt[:, 0:3, sq_of(3)], xt_r[:, 0:3, sq_of(3)])
            nc.gpsimd.dma_start(xt[:, 3:6, sq_of(3)], xt_r[:, 3:6, sq_of(3)])

            ktp = persist.tile([P, MC, NK], bf16, tag="ktp")
            qt = persist.tile([P, MC, S], bf16, tag="qt")

            # staging rows for the batched 1/sum: rows 0 (q=0) and 32 (q=1);
            # rows 1..31 must be finite for the batched Ln -> memset once
            sums_a = persist.tile([P, SQT], f32, tag="sums_a")
            sums_b = persist.tile([P, SQT], f32, tag="sums_b")
            nc.gpsimd.memset(sums_a[0:33, :], 1.0)
            nc.gpsimd.memset(sums_b[0:33, :], 1.0)
            sums_ab = [sums_a, sums_b]

            # ---- Q projection of tile t (m-outer, one PSUM bank) ----
            def q_proj(t, m):
                sq = slice(t * SQT, (t + 1) * SQT)
                ps = ps_cx.tile([P, SQT], f32, tag="cx", name=f"qp{t}_{m}")
                for i in range(KC):
                    nc.tensor.matmul(
                        ps,
                        lhsT=wqt[:, i, m * P : (m + 1) * P],
                        rhs=xt[:, i, sq],
                        start=(i == 0),
                        stop=(i == KC - 1),
                    )
                nc.vector.tensor_copy(qt[:, m, sq], ps)

            # Qp(0) first: it gates scores(pair 0) and nothing blocks it
            for m in range(MC):
                q_proj(0, m)

            # ---- K projection, i-outer ----
            kps0 = ps_sc.tile([P, 2, SQT], f32, tag="sc", name="kps0")
            kps1 = ps_cx.tile([P, SQT], f32, tag="cx", name="kps1")
            for i in range(KC):
                st, sp = i == 0, i == KC - 1
                nc.tensor.matmul(kps0[:, 0, :], lhsT=wkt[:, i, 0:P], rhs=kselt[:, i, :], start=st, stop=sp)
                nc.tensor.matmul(kps0[:, 1, :], lhsT=wkt[:, i, P : 2 * P], rhs=kselt[:, i, :], start=st, stop=sp)
                nc.tensor.matmul(kps1, lhsT=wkt[:, i, 2 * P : 3 * P], rhs=kselt[:, i, :], start=st, stop=sp)
            nc.vector.tensor_copy(ktp[:, 0, :], kps0[:, 0, :])
            nc.vector.tensor_copy(ktp[:, 1, :], kps0[:, 1, :])
            nc.vector.tensor_copy(ktp[:, 2, :], kps1)

            # ---- V projection, i-outer, emitted as a slot-0 filler so late
            # vselt never head-blocks the score/exp pipeline ----
            def emit_vproj():
                vps1 = ps_cx.tile([P, SQT], f32, tag="cx", name="vps1")
                vps2 = ps_cx.tile([P, SQT], f32, tag="cx", name="vps2")
                vps3 = ps_cx.tile([P, SQT], f32, tag="cx", name="vps3")
                vps4 = ps_pj.tile([P, SQT], f32, tag="pj", name="vps4")
                vtgt = [vps1[:, 0:DG], vps2[:, 0:DG], vps3[:, 0:DG], vps4[:, 0:DG]]
                for i in range(KC):
                    st, sp = i == 0, i == KC - 1
                    for c in range(SC):
                        nc.tensor.matmul(
                            vtgt[c],
                            lhsT=vselt[:, i, c * P : (c + 1) * P],
                            rhs=wvt[:, i, :],
                            start=st,
                            stop=sp,
                        )
                for c in range(SC):
                    nc.vector.tensor_copy(
                        vb[:, c, :, :, 0:HD],
                        vtgt[c].rearrange("p (j q d) -> p j q d", j=MC, q=2),
                    )

            # ---- steady state: pipeline over pair-slots Pidx = 3t + j ----
            NP = NSQT * MC  # 12 pair slots
            ep_of = {}
            ctxt_of = {}
            obuf_of = {}

            def emit_scores(Pidx):
                t, j = Pidx // MC, Pidx % MC
                sq = slice(t * SQT, (t + 1) * SQT)
                ept = ep_pool.tile([P, SC, 2, SQT], bf16, tag="ep", name=f"ep{Pidx}")
                ep_of[Pidx] = ept
                for ch in range(SC):  # one 2-bank tile per c-chunk
                    sc = ps_sc.tile([P, 2, SQT], f32, tag="sc", name=f"sc{Pidx}_{ch}")
                    for q in range(2):
                        nc.tensor.matmul(
                            sc[:, q, :],
                            lhsT=ktp[64 * q : 64 * q + 64, j, ch * P : (ch + 1) * P],
                            rhs=qt[64 * q : 64 * q + 64, j, sq],
                            start=True,
                            stop=True,
                        )
                    nc.scalar.activation(out=ept[:, ch, :, :], in_=sc, func=EXP)
                    yield ch

            pc_of = {}

            def emit_ctx_half(Pp, q):
                # ctx' matmul for head (pair jp, parity q); M=65: rows 0:64
                # ctx, row 64 = sum of exp over unmasked keys
                if not (0 <= Pp < NP):
                    return
                tp, jp = Pp // MC, Pp % MC
                if jp == 0 and q == 0:
                    ctxt_of[tp] = cx_pool.tile([P, MC, SQT], bf16, tag="ctxt", name=f"ctxt{tp}")
                ept = ep_of[Pp]
                pc = ps_cx.tile([P, SQT], f32, tag="cx", name=f"cx{Pp}_{q}")
                pc_of[(Pp, q)] = pc
                for c in range(SC):
                    nc.tensor.matmul(
                        pc[: HD + 1, :],
                        lhsT=vb[:, c, jp, q, 0 : HD + 1],
                        rhs=ept[:, c, q, :],
                        start=(c == 0),
                        stop=(c == SC - 1),
                    )
                nc.vector.tensor_copy(
                    sums_ab[Pp % 2][32 * q : 32 * q + 1, :], pc[HD : HD + 1, :]
                )
                if DEBUG and Pp == 0 and q == 1:
                    nc.sync.dma_start(dbg_ep_d[:, :, :, :], ep_of[0])

            rs_of = {}

            def emit_norm_act(Pp):
                # batched 1/sums = exp(-ln(sums)) for the pair's two heads
                # (rows 0 and 32); emitted between the two exp ACTs so it
                # fills the scalar queue while the PE refills score banks
                if not (0 <= Pp < NP):
                    return
                s2 = sums_ab[Pp % 2]
                ls = nrm_pool.tile([P, SQT], f32, tag="ls", name=f"ls{Pp}")
                rs = nrm_pool.tile([P, SQT], f32, tag="rs", name=f"rs{Pp}")
                nc.scalar.activation(out=ls[0:33, :], in_=s2[0:33, :], func=LN)
                nc.scalar.activation(out=rs[0:33, :], in_=ls[0:33, :], func=EXP, scale=-1.0)
                rs_of[Pp] = rs

            def emit_norm_finish(Pp):
                # gpsimd partition_broadcast replicates each head's 1/sum row
                # to 64 SBUF rows (src must sit at absolute partition 0: q=1's
                # row 32 is DMA'd down first, from gpsimd's own queue so the
                # chain never touches the sync engine); then one fused DVE
                # multiply+cast produces ctxt
                if not (0 <= Pp < NP):
                    return
                tp, jp = Pp // MC, Pp % MC
                rs = rs_of.pop(Pp)
                for q in range(2):
                    pc = pc_of.pop((Pp, q))
                    if q == 0:
                        rrow = rs[0:1, :]
                    else:
                        r1 = nrm_pool.tile([1, SQT], f32, tag="r1", name=f"r1_{Pp}")
                        nc.sync.dma_start(r1, rs[32:33, :])
                        rrow = r1
                    rb = nrm_pool.tile([HD, SQT], f32, tag="rb", name=f"rb{Pp}_{q}")
                    nc.gpsimd.partition_broadcast(rb, rrow)
                    nc.vector.tensor_mul(
                        ctxt_of[tp][64 * q : 64 * q + 64, jp, :],
                        pc[0:HD, :],
                        rb[0:HD, :],
                    )

            def emit_outproj_chunk(tp, mq, n, use_sc=False):
                sq0 = tp * SQT + mq * P
                if n == 0:
                    obuf_of[(tp, mq)] = ob_pool.tile([P, D], bf16, tag="ob", name=f"ob{tp}_{mq}")
                if use_sc:
                    # drain slots: the score pool is idle, borrow it so the
                    # tail out-projections double-buffer
                    ps = ps_sc.tile([P, 2, SQT], f32, tag="sc", name=f"op{tp}_{mq}_{n}")[:, 0, :]
                else:
                    ps = ps_pj.tile([P, SQT], f32, tag="pj", name=f"op{tp}_{mq}_{n}")
                for jj in range(MC):
                    nc.tensor.matmul(
                        ps[:, 0:DG],
                        lhsT=ctxt_of[tp][:, jj, mq * P : (mq + 1) * P],
                        rhs=wot[:, jj, n * DG : (n + 1) * DG],
                        start=(jj == 0),
                        stop=(jj == MC - 1),
                    )
                nc.vector.tensor_copy(obuf_of[(tp, mq)][:, n * DG : (n + 1) * DG], ps[:, 0:DG])
                if n == 1:
                    nc.gpsimd.dma_start(out_d[sq0 : sq0 + P, :], obuf_of[(tp, mq)])

            # filler schedule: position -> list of work closures
            filler = {Pi: [] for Pi in range(NP + 5)}
            filler[0].append(("vp",))
            for t in range(NSQT - 1):
                for m in range(MC):
                    filler[MC * t + m].append(("qp", t + 1, m))
            for tp in range(NSQT):
                # tile 3's chunks run in the pure drain: start them one slot
                # earlier (right after its last norm_finish) - no steady-state
                # PE contention there, unlike the earlier global-shift attempt
                base = MC * tp + (4 if tp == NSQT - 1 else 5)
                filler[base].extend([("op", tp, 0, 0), ("op", tp, 0, 1), ("op", tp, 1, 0)])
                filler[base + 1].extend([("op", tp, 1, 1), ("op", tp, 2, 0), ("op", tp, 2, 1)])
                filler[base + 2].extend([("op", tp, 3, 0), ("op", tp, 3, 1)])

            def run_filler(Pidx):
                for item in filler.get(Pidx, []):
                    if item[0] == "qp":
                        q_proj(item[1], item[2])
                    elif item[0] == "vp":
                        emit_vproj()
                    else:
                        emit_outproj_chunk(item[1], item[2], item[3], use_sc=Pidx >= NP)

            for Pidx in range(NP + 5):
                if Pidx < NP:
                    gen = emit_scores(Pidx)
                    next(gen)  # ch 0
                    next(gen)  # ch 1
                    emit_ctx_half(Pidx - 2, 0)
                    next(gen)  # ch 2
                    emit_ctx_half(Pidx - 2, 1)
                    emit_norm_act(Pidx - 2)
                    for _ in gen:  # ch 3
                        pass
                    emit_norm_finish(Pidx - 2)
                else:
                    emit_ctx_half(Pidx - 2, 0)
                    emit_ctx_half(Pidx - 2, 1)
                    emit_norm_act(Pidx - 2)
                    emit_norm_finish(Pidx - 2)
                run_filler(Pidx)
                if DEBUG and Pidx == NP + 3:
                    nc.sync.dma_start(dbg_xt_d[:, :, :], xt)
                if DEBUG and Pidx == 4:
                    nc.sync.dma_start(dbg_ktp_d[:, :, :], ktp)
                    nc.sync.dma_start(dbg_qt_d[:, :, :], qt)
                    nc.sync.dma_start(dbg_vb_d[:, :, :, :, :], vb)
                    nc.sync.dma_start(dbg_ctxt_d[:, :, :], ctxt_of[0])

    # Pin Exp and Ln to the one table set that holds both so the scalar
    # engine never reloads activation tables when alternating exp(scores)
    # with the ln/exp reciprocal.
    _orig_tables = bacc.get_activation_tables

    def _pinned_tables(arch):
        tabs = {k: set(v) for k, v in _orig_tables(arch).items()}
        for name, fns in tabs.items():
            if name != "natural_log_exp_and_others":
                fns.discard(EXP)
                fns.discard(LN)
        return tabs

    bacc.get_activation_tables = _pinned_tables
    try:
        nc.compile()
    finally:
        bacc.get_activation_tables = _orig_tables
    return nc


def _get_nc():
    if "nc" not in _CACHE:
        _CACHE["nc"] = _build_bass()
    return _CACHE["nc"]


def kernel(query, key, value, mask, uniform_set, Wq, bq, Wk, bk, Wv, bv, Wo, bo):
    import ml_dtypes
    from concourse import bass_utils

    bft = ml_dtypes.bfloat16

    query = np.asarray(query, dtype=np.float32)
    key = np.asarray(key, dtype=np.float32)
    value = np.asarray(value, dtype=np.float32)
    mask = np.asarray(mask, dtype=np.float32)
    us = np.asarray(uniform_set).astype(bool)
    Wq = np.asarray(Wq, dtype=np.float32)
    Wk = np.asarray(Wk, dtype=np.float32)
    Wv = np.asarray(Wv, dtype=np.float32)
    Wo = np.asarray(Wo, dtype=np.float32)
    bq = np.asarray(bq, dtype=np.float32)
    bk = np.asarray(bk, dtype=np.float32)
    bv = np.asarray(bv, dtype=np.float32)
    bo = np.asarray(bo, dtype=np.float32)
    assert np.all(bq == 0.0), "kernel assumes bq == 0 (reference generates zeros)"

    nc = _get_nc()

    scale = 1.0 / float(HD) ** 0.5
    wqt_g = [np.ascontiguousarray((Wq.T[:, g * DG : (g + 1) * DG] * scale)).astype(bft) for g in range(HG)]
    wkt_g = [np.ascontiguousarray(Wk.T[:, g * DG : (g + 1) * DG]).astype(bft) for g in range(HG)]
    wvt_g = [np.ascontiguousarray(Wv.T[:, g * DG : (g + 1) * DG]).astype(bft) for g in range(HG)]
    wot_g = [np.ascontiguousarray(Wo.T[g * DG : (g + 1) * DG, :]).astype(bft) for g in range(HG)]

    in_maps = []
    for b in range(B):
        keep = us & (mask[b, 0, 0] >= 0)
        idx = np.nonzero(keep)[0]
        n = len(idx)
        assert 0 < n <= NK, f"selected key count {n} unsupported"
        kselt = np.zeros((D, NK), bft)
        kselt[:, :n] = key[b][idx].T.astype(bft)
        vselt = np.zeros((D, NK), bft)
        vselt[:, :n] = value[b][idx].T.astype(bft)
        kmask = np.zeros((NK,), np.float32)
        kmask[:n] = 1.0
        # [SC, P, HPG, HD]: key k = c*128 + p, replicated over (head, hd)
        kmask64 = np.ascontiguousarray(
            np.broadcast_to(kmask.reshape(SC, P, 1, 1), (SC, P, HPG, 1))
        ).astype(bft)
        xt = np.ascontiguousarray(query[b].T).astype(bft)
        for g in range(HG):
            in_maps.append(
                {
                    "xt": xt,
                    "kselt": kselt,
                    "vselt": vselt,
                    "wqt": wqt_g[g],
                    "wkt": wkt_g[g],
                    "wvt": wvt_g[g],
                    "wot": wot_g[g],
                    "kmask64": kmask64,
                }
            )

    res = bass_utils.run_bass_kernel_spmd(nc, in_maps, core_ids=list(range(B * HG)))
    outs = [np.asarray(m["out"], dtype=np.float32) for m in res.results]

    corr = (bo + Wo @ bv).astype(np.float32)
    out = np.empty((B, S, D), np.float32)
    for b in range(B):
        out[b] = outs[HG * b] + outs[HG * b + 1] + corr
    return out
